# revision 19
# baseline (speedup 1.0000x reference)
"""Trainium2 Bass kernel for CausalSelfAttention (B=4, T=2048, C=2048, H=16).

Sharding: 8 cores = 4 batches x 2 head-groups (8 heads each).
Each core computes q/k/v projections for its heads, RoPE, causal attention,
and a partial output projection (row-parallel c_proj over its heads' columns).
Host sums the two partials per batch (standard row-parallel TP unshard).

On-chip layout notes:
  - All matmul contractions run with the contracted dim on partitions.
  - Host pre-transposes x and weights so every DMA is contiguous.
  - Scores are computed transposed (s^T[tk, tq]); softmax denominator is
    summed on DVE then partition-all-reduced on GpSimd (which also yields
    the partition broadcast for free), so the PE does no reduction work.
  - Biases are added on DVE from partition-replicated SBUF tables (no
    ones-matmuls on the PE).
  - RoPE rotate-half is a fixed 128x128 signed permutation applied via one
    extra matmul per q/k tile; cos/sin enter as elementwise tables.
  - Emission is software-pipelined: the attention head loop of chunk j is
    interleaved with chunk j+1's projections and chunk j-1's output
    projection, so the PE never starves while the scalar engine works
    through the exp() stream (exp is the per-head pacing limit).
"""

import numpy as np
import ml_dtypes

import concourse.bass as bass
import concourse.mybir as mybir
import concourse.tile as tile
from concourse import bacc
from concourse import bass_isa
from concourse.alu_op_type import AluOpType
from concourse.bass import ds
from concourse.bass_utils import run_bass_kernel_spmd

BF16 = ml_dtypes.bfloat16
F32 = np.float32

B = 4
C = 2048
H = 16
D = 128
HPC = 8          # heads per core
P = 128
CH = 512         # tq chunk width
NCT = C // P     # 16 contraction tiles
AF = mybir.ActivationFunctionType
SCALE = 1.0 / float(np.sqrt(np.float32(D)))


def build_nc(T=2048):
    NCH = T // CH
    dt = mybir.dt
    nc = bacc.Bacc(None, target_bir_lowering=False)

    xT = nc.dram_tensor("xT", [C, T], dt.bfloat16, kind="ExternalInput")
    wq = nc.dram_tensor("wq", [C, HPC * D], dt.bfloat16, kind="ExternalInput")
    wk = nc.dram_tensor("wk", [C, HPC * D], dt.bfloat16, kind="ExternalInput")
    wv = nc.dram_tensor("wv", [C, HPC * D], dt.bfloat16, kind="ExternalInput")
    wp = nc.dram_tensor("wp", [HPC * D, C], dt.bfloat16, kind="ExternalInput")
    ab_a = nc.dram_tensor("ab_a", [D, T], dt.bfloat16, kind="ExternalInput")
    ab_b = nc.dram_tensor("ab_b", [D, T], dt.bfloat16, kind="ExternalInput")
    bq = nc.dram_tensor("bq", [D, HPC], dt.float32, kind="ExternalInput")
    bk = nc.dram_tensor("bk", [D, HPC], dt.float32, kind="ExternalInput")
    bv = nc.dram_tensor("bv", [1, HPC * D], dt.bfloat16, kind="ExternalInput")
    bp = nc.dram_tensor("bp", [1, C], dt.bfloat16, kind="ExternalInput")
    maskd = nc.dram_tensor("maskd", [P, D], dt.bfloat16, kind="ExternalInput")
    pt = nc.dram_tensor("pt", [D, D], dt.bfloat16, kind="ExternalInput")
    out = nc.dram_tensor("out", [T, C], dt.float32, kind="ExternalOutput")

    xT_r = xT.rearrange("(ct p) t -> p ct t", p=P)
    wq_r = wq.rearrange("(ct p) d -> p ct d", p=P)
    wk_r = wk.rearrange("(ct p) d -> p ct d", p=P)
    wv_r = wv.rearrange("(ct p) d -> p ct d", p=P)
    wp_r = wp.rearrange("(hc p) o -> p hc o", p=P)

    with tile.TileContext(nc) as tc:
        with (
            tc.tile_pool(name="consts", bufs=1) as consts,
            tc.tile_pool(name="keep", bufs=1) as keep,
            tc.tile_pool(name="xw", bufs=2) as xwp,
            tc.tile_pool(name="wtp", bufs=2) as wtp,
            tc.tile_pool(name="wpp", bufs=2) as wpp,
            tc.tile_pool(name="work", bufs=3) as work,
            tc.tile_pool(name="qpp", bufs=14) as qpp,
            tc.tile_pool(name="ycp", bufs=2) as ycp,
            tc.tile_pool(name="denp", bufs=2) as denp,
            tc.tile_pool(name="outp", bufs=2) as outp,
            tc.tile_pool(name="ps_acc", bufs=2, space="PSUM") as ps_acc,
            tc.tile_pool(name="ps_misc", bufs=2, space="PSUM") as ps_misc,
            tc.tile_pool(name="ps_s", bufs=2, space="PSUM") as ps_s,
            tc.tile_pool(name="ps_y", bufs=2, space="PSUM") as ps_y,
        ):
            mask_sb = consts.tile([P, D], dt.bfloat16)
            pt_sb = consts.tile([D, D], dt.bfloat16)
            bq_sb = consts.tile([D, HPC], dt.float32)
            bk_sb = consts.tile([D, HPC], dt.float32)
            bv_rep = consts.tile([P, HPC * D], dt.bfloat16)
            bp_rep = consts.tile([P, C], dt.bfloat16)
            wv_sb = consts.tile([P, NCT, HPC * D], dt.bfloat16)

            nc.sync.dma_start(out=mask_sb, in_=maskd[:])
            nc.sync.dma_start(out=pt_sb, in_=pt[:])
            nc.sync.dma_start(out=bq_sb, in_=bq[:])
            nc.sync.dma_start(out=bk_sb, in_=bk[:])

            kT = keep.tile([P, HPC, T], dt.bfloat16)
            vS = keep.tile([P, HPC, T], dt.bfloat16)

            # per-chunk state (filled by the emission closures below)
            xc_t = {}      # j -> x chunk tile
            ab_t = {}      # j -> (a_sb, b_sb)
            qp_t = {}      # j -> list of q tiles
            yc_t = {}      # j -> y chunk tile [P, HPC, CH]
            rope_pend = {}  # j -> deferred (raw, dest) for the rope pipeline

            def load_chunk_inputs(j):
                cols = ds(j * CH, CH)
                xc = xwp.tile([P, NCT, CH], dt.bfloat16, tag="xc")
                for cg in range(4):
                    nc.sync.dma_start(
                        out=xc[:, ds(cg * 4, 4), :],
                        in_=xT_r[:, ds(cg * 4, 4), cols],
                    )
                a_sb = work.tile([D, CH], dt.bfloat16, tag="abA", bufs=2)
                nc.sync.dma_start(out=a_sb, in_=ab_a[:, cols])
                b_sb = work.tile([D, CH], dt.bfloat16, tag="abB", bufs=2)
                nc.sync.dma_start(out=b_sb, in_=ab_b[:, cols])
                xc_t[j] = xc
                ab_t[j] = (a_sb, b_sb)
                qp_t[j] = []
                rope_pend[j] = None

            def emit_rope(j, raw, dest):
                # q'/k' = A (.) raw + B (.) (P @ raw), via one PE matmul for
                # the rotate-half permutation
                a_sb, b_sb = ab_t[j]
                rps = ps_misc.tile([P, CH], dt.float32, tag="rot")
                nc.tensor.matmul(rps, lhsT=pt_sb, rhs=raw, start=True, stop=True)
                t1 = work.tile([P, CH], dt.bfloat16, tag="t1", bufs=2)
                nc.gpsimd.tensor_tensor(out=t1, in0=raw, in1=a_sb, op=AluOpType.mult)
                t2 = work.tile([P, CH], dt.bfloat16, tag="t2", bufs=2)
                nc.vector.tensor_tensor(out=t2, in0=rps, in1=b_sb, op=AluOpType.mult)
                nc.vector.tensor_tensor(out=dest, in0=t1, in1=t2, op=AluOpType.add)

            def qk_tile(j, qk, h):
                # one head's q or k projection (+ pipelined rope of the
                # previous tile)
                cols = ds(j * CH, CH)
                wsrc = wq_r if qk == 0 else wk_r
                bsrc = bq_sb if qk == 0 else bk_sb
                xc = xc_t[j]
                ps = ps_acc.tile([P, CH], dt.float32, tag="acc")
                for cth in range(2):
                    wt = wtp.tile([P, NCT // 2, D], dt.bfloat16, tag="wt", name="wt")
                    nc.sync.dma_start(
                        out=wt, in_=wsrc[:, ds(cth * 8, 8), ds(h * D, D)]
                    )
                    for c in range(NCT // 2):
                        ct = cth * 8 + c
                        nc.tensor.matmul(
                            ps,
                            lhsT=wt[:, c, :],
                            rhs=xc[:, ct, :],
                            start=(ct == 0),
                            stop=(ct == NCT - 1),
                        )
                raw = work.tile([P, CH], dt.bfloat16, tag="raw", bufs=2)
                nc.vector.tensor_tensor(
                    out=raw,
                    in0=ps,
                    in1=bsrc[:, ds(h, 1)].to_broadcast([P, CH]),
                    op=AluOpType.add,
                )
                if qk == 0:
                    dest = qpp.tile([P, CH], dt.bfloat16, tag="qp")
                    qp_t[j].append(dest)
                else:
                    dest = kT[:, h, cols]
                if rope_pend[j] is not None:
                    emit_rope(j, *rope_pend[j])
                rope_pend[j] = (raw, dest)

            def qk_flush(j):
                if rope_pend[j] is not None:
                    emit_rope(j, *rope_pend[j])
                    rope_pend[j] = None

            def v_group(j, half, tt):
                # one [t-tile x half-of-heads] v projection group, bias
                # fused into the DVE copy out of PSUM
                xc = xc_t[j]
                ps = ps_acc.tile([P, CH], dt.float32, tag="acc")
                for ct in range(NCT):
                    nc.tensor.matmul(
                        ps,
                        lhsT=xc[:, ct, ds(tt * D, D)],
                        rhs=wv_sb[:, ct, ds(half * CH, CH)],
                        start=(ct == 0),
                        stop=(ct == NCT - 1),
                    )
                ti = 4 * j + tt
                nc.vector.tensor_tensor(
                    out=vS[:, ds(half * 4, 4), ds(ti * D, D)],
                    in0=ps[:, :].rearrange("p (a b) -> p a b", b=D),
                    in1=bv_rep[:, ds(half * CH, CH)].rearrange(
                        "p (a b) -> p a b", b=D
                    ),
                    op=AluOpType.add,
                )

            def att_head(j, h):
                if h == 0:
                    yc_t[j] = ycp.tile(
                        [P, HPC, CH], dt.bfloat16, tag="yc", name="yc"
                    )
                qp = qp_t[j][h]
                den_a = denp.tile([P, CH], dt.float32, tag="dena")
                yps = ps_y.tile([P, CH], dt.float32, tag="y")
                ntk = 4 * (j + 1)
                exq = []  # (ex, i, off) pending y-matmuls
                for i in range(ntk):
                    sps = ps_s.tile([P, CH], dt.float32, tag="s")
                    m = i - 4 * j
                    off = max(m, 0) * D  # valid tq cols start here
                    w = CH - off
                    nc.tensor.matmul(
                        sps[:, ds(off, w)],
                        lhsT=kT[:, h, ds(i * D, D)],
                        rhs=qp[:, ds(off, w)],
                        start=True,
                        stop=True,
                    )
                    ex = work.tile([P, CH], dt.bfloat16, tag="ex", bufs=3)
                    nc.scalar.activation(
                        ex[:, ds(off, w)], sps[:, ds(off, w)],
                        AF.Exp, scale=SCALE,
                    )
                    if m >= 0:
                        # triangular mask on the diagonal 128-block
                        nc.vector.tensor_tensor(
                            out=ex[:, ds(off, D)],
                            in0=ex[:, ds(off, D)],
                            in1=mask_sb,
                            op=AluOpType.mult,
                        )
                    if i == 0:
                        nc.vector.tensor_copy(
                            out=den_a[:, ds(off, w)], in_=ex[:, ds(off, w)]
                        )
                        if off > 0:
                            nc.vector.memset(den_a[:, ds(0, off)], 0.0)
                    else:
                        nc.vector.tensor_tensor(
                            out=den_a[:, ds(off, w)],
                            in0=den_a[:, ds(off, w)],
                            in1=ex[:, ds(off, w)],
                            op=AluOpType.add,
                        )
                    exq.append((ex, i, off))
                    if len(exq) > 2:
                        pex, pi, poff = exq.pop(0)
                        nc.tensor.matmul(
                            yps[:, ds(poff, CH - poff)],
                            lhsT=vS[:, h, ds(pi * D, D)],
                            rhs=pex[:, ds(poff, CH - poff)],
                            start=(pi == 0),
                            stop=False,
                        )
                # start the denominator all-reduce on GpSimd while the PE
                # drains the pending att@v matmuls
                den_bc = denp.tile([P, CH], dt.float32, tag="dbc")
                nc.gpsimd.partition_all_reduce(
                    den_bc, den_a, channels=P, reduce_op=bass_isa.ReduceOp.add
                )
                while exq:
                    pex, pi, poff = exq.pop(0)
                    nc.tensor.matmul(
                        yps[:, ds(poff, CH - poff)],
                        lhsT=vS[:, h, ds(pi * D, D)],
                        rhs=pex[:, ds(poff, CH - poff)],
                        start=(pi == 0),
                        stop=(not exq),
                    )
                # DVE frees the y PSUM bank first; the reciprocal (which
                # waits on GpSimd) comes after so it never delays the copy,
                # and the final normalize runs on GpSimd off the DVE path.
                yraw = work.tile([P, CH], dt.bfloat16, tag="yraw", bufs=2)
                nc.vector.tensor_copy(out=yraw, in_=yps)
                nc.vector.reciprocal(den_bc, den_bc)
                nc.gpsimd.tensor_tensor(
                    out=yc_t[j][:, h, :], in0=yraw, in1=den_bc, op=AluOpType.mult
                )

            def oproj_piece(j, oc):
                yc = yc_t[j]
                wpc = []
                for hg in range(2):
                    w = wpp.tile([P, HPC // 2, CH], dt.bfloat16, tag="wpc",
                                 name="wpc", bufs=3)
                    nc.sync.dma_start(
                        out=w, in_=wp_r[:, ds(hg * 4, 4), ds(oc * CH, CH)]
                    )
                    wpc.append(w)
                for ttl in range(4):
                    ps = ps_misc.tile([P, CH], dt.float32, tag="rot")
                    for hc in range(HPC):
                        nc.tensor.matmul(
                            ps,
                            lhsT=yc[:, hc, ds(ttl * D, D)],
                            rhs=wpc[hc // 4][:, hc % 4, :],
                            start=(hc == 0),
                            stop=(hc == HPC - 1),
                        )
                    ot = outp.tile([P, CH], dt.float32, tag="ot")
                    nc.vector.tensor_tensor(
                        out=ot, in0=ps, in1=bp_rep[:, ds(oc * CH, CH)],
                        op=AluOpType.add,
                    )
                    nc.sync.dma_start(
                        out=out[ds((4 * j + ttl) * P, P), ds(oc * CH, CH)],
                        in_=ot,
                    )

            def proj_fillers(j):
                # list of closures emitting chunk j's projections piecewise
                fs = []
                for qk in range(2):
                    for h in range(HPC):
                        fs.append(lambda j=j, qk=qk, h=h: qk_tile(j, qk, h))
                fs.append(lambda j=j: qk_flush(j))
                for half in range(2):
                    for tt in range(4):
                        fs.append(
                            lambda j=j, half=half, tt=tt: v_group(j, half, tt)
                        )
                return fs

            # ---- prologue: chunk 0 inputs + projections ----
            # big constant / weight loads are emitted after the first few
            # projection tiles so the first matmuls aren't queued behind
            # them on the DMA rings
            load_chunk_inputs(0)
            p0 = proj_fillers(0)
            for f in p0[:4]:
                f()
            if NCH > 1:
                load_chunk_inputs(1)
            nc.sync.dma_start(
                out=bv_rep, in_=bv[0][None, :].to_broadcast([P, HPC * D])
            )
            nc.sync.dma_start(
                out=bp_rep, in_=bp[0][None, :].to_broadcast([P, C])
            )
            for cg in range(4):
                nc.sync.dma_start(
                    out=wv_sb[:, ds(cg * 4, 4), :],
                    in_=wv_r[:, ds(cg * 4, 4), :],
                )
            for f in p0[4:]:
                f()

            # ---- main software-pipelined loop ----
            for j in range(NCH):
                fillers = []
                if j + 2 < NCH:
                    fillers.append(lambda j=j: load_chunk_inputs(j + 2))
                if j + 1 < NCH:
                    fillers += proj_fillers(j + 1)
                if j >= 1:
                    # spread the out-projection pieces through the filler list
                    step = max(1, len(fillers) // 4)
                    for oc in range(C // CH):
                        fillers.insert(
                            min((oc + 1) * step + oc, len(fillers)),
                            lambda j=j, oc=oc: oproj_piece(j - 1, oc),
                        )
                per = (len(fillers) + HPC - 1) // HPC
                fi = 0
                for h in range(HPC):
                    att_head(j, h)
                    for _ in range(per):
                        if fi < len(fillers):
                            fillers[fi]()
                            fi += 1
                while fi < len(fillers):
                    fillers[fi]()
                    fi += 1

            # ---- epilogue: last chunk's output projection ----
            for oc in range(C // CH):
                oproj_piece(NCH - 1, oc)

    nc.compile()
    return nc


def _rope_tables(T):
    inv_freq = (
        1.0 / (10000.0 ** (np.arange(0, D, 2, dtype=np.float32) / np.float32(D)))
    ).astype(np.float32)
    t = np.arange(T, dtype=np.float32)
    freqs = t[:, None] * inv_freq[None, :]
    emb = np.concatenate((freqs, freqs), axis=-1)
    cos = np.cos(emb).astype(np.float32)
    sin = np.sin(emb).astype(np.float32)
    A = np.ascontiguousarray((cos + sin).T).astype(BF16)
    Bt = np.ascontiguousarray((cos - sin).T).astype(BF16)
    return A, Bt


def _rot_pt():
    Pm = np.zeros((D, D), dtype=np.float32)
    for d in range(64):
        Pm[d, 2 * d + 1] = -1.0
        Pm[64 + d, 2 * d] = 1.0
    return np.ascontiguousarray(Pm.T).astype(BF16)


def _maskd():
    # maskd[p, c] = 0 where tq < tk within a diagonal 128x128 block
    row = np.arange(P)[:, None]
    col = np.arange(D)[None, :]
    return np.where(col < row, 0.0, 1.0).astype(BF16)


def make_in_maps(x, w_attn, b_attn, w_proj, b_proj, T=2048):
    A, Bt = _rope_tables(T)
    pt = _rot_pt()
    maskd = _maskd()
    in_maps = []
    for core in range(8):
        b, g = core // 2, core % 2
        gs = slice(g * 1024, (g + 1) * 1024)
        bp_eff = b_proj if g == 0 else np.zeros_like(b_proj)
        in_maps.append(
            {
                "xT": np.ascontiguousarray(x[b][:T].T).astype(BF16),
                "wq": np.ascontiguousarray(w_attn[gs, :].T).astype(BF16),
                "wk": np.ascontiguousarray(w_attn[2048:4096][gs, :].T).astype(BF16),
                "wv": np.ascontiguousarray(w_attn[4096:6144][gs, :].T).astype(BF16),
                "wp": np.ascontiguousarray(w_proj[:, gs].T).astype(BF16),
                "ab_a": A,
                "ab_b": Bt,
                "bq": np.ascontiguousarray(
                    b_attn[gs].reshape(HPC, D).T
                ).astype(np.float32),
                "bk": np.ascontiguousarray(
                    b_attn[2048:4096][gs].reshape(HPC, D).T
                ).astype(np.float32),
                "bv": b_attn[4096:6144][gs].reshape(1, HPC * D).astype(BF16),
                "bp": bp_eff.reshape(1, C).astype(BF16),
                "maskd": maskd,
                "pt": pt,
            }
        )
    return in_maps


_NC_CACHE = {}


def run(x, w_attn, b_attn, w_proj, b_proj, trace=False, trace_cores=None):
    T = x.shape[1]
    if T not in _NC_CACHE:
        _NC_CACHE[T] = build_nc(T)
    nc = _NC_CACHE[T]
    in_maps = make_in_maps(
        np.asarray(x, dtype=np.float32),
        np.asarray(w_attn, dtype=np.float32),
        np.asarray(b_attn, dtype=np.float32),
        np.asarray(w_proj, dtype=np.float32),
        np.asarray(b_proj, dtype=np.float32),
        T=T,
    )
    res = run_bass_kernel_spmd(
        nc, in_maps, core_ids=list(range(8)), trace=trace, trace_cores=trace_cores
    )
    T_, C_ = in_maps[0]["xT"].shape[1], C
    out = np.zeros((B, T_, C_), dtype=np.float32)
    for b in range(B):
        out[b] = res.results[2 * b]["out"] + res.results[2 * b + 1]["out"]
    return out, res


def kernel(x, w_attn, b_attn, w_proj, b_proj):
    out, _ = run(x, w_attn, b_attn, w_proj, b_proj, trace=False)
    return out


# revision 24
# speedup vs baseline: 1.1246x; 1.1246x over previous
"""Trainium2 Bass kernel for CausalSelfAttention (B=4, T=2048, C=2048, H=16).

Sharding: 8 cores = 4 batches x 2 head-groups (8 heads each).
Each core computes q/k/v projections for its heads, RoPE, causal attention,
and a partial output projection (row-parallel c_proj over its heads' columns).
Host sums the two partials per batch (standard row-parallel TP unshard).

On-chip layout notes:
  - All matmul contractions run with the contracted dim on partitions.
  - Host pre-transposes x and weights so every DMA is contiguous.
  - Scores are computed transposed (s^T[tk, tq]); softmax denominator is
    summed on DVE then partition-all-reduced on GpSimd (which also yields
    the partition broadcast for free), so the PE does no reduction work.
  - Biases are added on DVE from partition-replicated SBUF tables (no
    ones-matmuls on the PE).
  - RoPE rotate-half is a fixed 128x128 signed permutation applied via one
    extra matmul per q/k tile; cos/sin enter as elementwise tables.
  - Emission is software-pipelined: the attention head loop of chunk j is
    interleaved with chunk j+1's projections and chunk j-1's output
    projection, so the PE never starves while the scalar engine works
    through the exp() stream (exp is the per-head pacing limit).
"""

import numpy as np
import ml_dtypes

import concourse.bass as bass
import concourse.mybir as mybir
import concourse.tile as tile
from concourse import bacc
from concourse import bass_isa
from concourse.alu_op_type import AluOpType
from concourse.bass import ds
from concourse.bass_utils import run_bass_kernel_spmd

BF16 = ml_dtypes.bfloat16
F32 = np.float32

B = 4
C = 2048
H = 16
D = 128
HPC = 8          # heads per core
P = 128
CH = 512         # tq chunk width
NCT = C // P     # 16 contraction tiles
AF = mybir.ActivationFunctionType
SCALE = 1.0 / float(np.sqrt(np.float32(D)))


def build_nc(T=2048):
    NCH = T // CH
    dt = mybir.dt
    nc = bacc.Bacc(None, target_bir_lowering=False)

    xT = nc.dram_tensor("xT", [C, T], dt.bfloat16, kind="ExternalInput")
    wq = nc.dram_tensor("wq", [C, HPC * D], dt.bfloat16, kind="ExternalInput")
    wk = nc.dram_tensor("wk", [C, HPC * D], dt.bfloat16, kind="ExternalInput")
    wv = nc.dram_tensor("wv", [C, HPC * D], dt.bfloat16, kind="ExternalInput")
    wp = nc.dram_tensor("wp", [HPC * D, C], dt.bfloat16, kind="ExternalInput")
    ab_a = nc.dram_tensor("ab_a", [D, T], dt.bfloat16, kind="ExternalInput")
    ab_b = nc.dram_tensor("ab_b", [D, T], dt.bfloat16, kind="ExternalInput")
    bq = nc.dram_tensor("bq", [D, HPC], dt.float32, kind="ExternalInput")
    bk = nc.dram_tensor("bk", [D, HPC], dt.float32, kind="ExternalInput")
    bv = nc.dram_tensor("bv", [1, HPC * D], dt.bfloat16, kind="ExternalInput")
    bp = nc.dram_tensor("bp", [1, C], dt.bfloat16, kind="ExternalInput")
    maskd = nc.dram_tensor("maskd", [P, D], dt.bfloat16, kind="ExternalInput")
    pt = nc.dram_tensor("pt", [D, D], dt.bfloat16, kind="ExternalInput")
    onc = nc.dram_tensor("onc", [P, 1], dt.float32, kind="ExternalInput")
    out = nc.dram_tensor("out", [T, C], dt.float32, kind="ExternalOutput")

    xT_r = xT.rearrange("(ct p) t -> p ct t", p=P)
    wq_r = wq.rearrange("(ct p) d -> p ct d", p=P)
    wk_r = wk.rearrange("(ct p) d -> p ct d", p=P)
    wv_r = wv.rearrange("(ct p) d -> p ct d", p=P)
    wp_r = wp.rearrange("(hc p) o -> p hc o", p=P)

    with tile.TileContext(nc) as tc:
        with (
            tc.tile_pool(name="consts", bufs=1) as consts,
            tc.tile_pool(name="keep", bufs=1) as keep,
            tc.tile_pool(name="xw", bufs=2) as xwp,
            tc.tile_pool(name="wtp", bufs=2) as wtp,
            tc.tile_pool(name="wpp", bufs=2) as wpp,
            tc.tile_pool(name="work", bufs=3) as work,
            tc.tile_pool(name="qpp", bufs=14) as qpp,
            tc.tile_pool(name="ycp", bufs=2) as ycp,
            tc.tile_pool(name="denp", bufs=2) as denp,
            tc.tile_pool(name="outp", bufs=2) as outp,
            tc.tile_pool(name="ps_acc", bufs=2, space="PSUM") as ps_acc,
            tc.tile_pool(name="ps_misc", bufs=2, space="PSUM") as ps_misc,
            tc.tile_pool(name="ps_s", bufs=2, space="PSUM") as ps_s,
            tc.tile_pool(name="ps_y", bufs=2, space="PSUM") as ps_y,
        ):
            mask_sb = consts.tile([P, D], dt.bfloat16)
            pt_sb = consts.tile([D, D], dt.bfloat16)
            bq_sb = consts.tile([D, HPC], dt.float32)
            bk_sb = consts.tile([D, HPC], dt.float32)
            bv_rep = consts.tile([P, HPC * D], dt.bfloat16)
            bp_rep = consts.tile([P, C], dt.bfloat16)
            wv_sb = consts.tile([P, NCT, HPC * D], dt.bfloat16)

            onc_sb = consts.tile([P, 1], dt.float32)

            nc.sync.dma_start(out=mask_sb, in_=maskd[:])
            nc.sync.dma_start(out=pt_sb, in_=pt[:])
            nc.sync.dma_start(out=bq_sb, in_=bq[:])
            nc.sync.dma_start(out=bk_sb, in_=bk[:])
            nc.sync.dma_start(out=onc_sb, in_=onc[:])

            kT = keep.tile([P, HPC, T], dt.bfloat16)
            vS = keep.tile([P, HPC, T], dt.bfloat16)

            # per-chunk state (filled by the emission closures below)
            xc_t = {}      # j -> x chunk tile
            ab_t = {}      # j -> (a_sb, b_sb)
            qp_t = {}      # j -> list of q tiles
            yc_t = {}      # j -> y chunk tile [P, HPC, CH]
            rope_pend = {}  # j -> deferred (raw, dest) for the rope pipeline

            def load_chunk_inputs(j):
                cols = ds(j * CH, CH)
                xc = xwp.tile([P, NCT, CH], dt.bfloat16, tag="xc")
                for cg in range(4):
                    nc.sync.dma_start(
                        out=xc[:, ds(cg * 4, 4), :],
                        in_=xT_r[:, ds(cg * 4, 4), cols],
                    )
                a_sb = work.tile([D, CH], dt.bfloat16, tag="abA", bufs=2)
                nc.sync.dma_start(out=a_sb, in_=ab_a[:, cols])
                b_sb = work.tile([D, CH], dt.bfloat16, tag="abB", bufs=2)
                nc.sync.dma_start(out=b_sb, in_=ab_b[:, cols])
                xc_t[j] = xc
                ab_t[j] = (a_sb, b_sb)
                qp_t[j] = []
                rope_pend[j] = None

            def emit_rope(j, raw, dest):
                # q'/k' = A (.) raw + B (.) (P @ raw), via one PE matmul for
                # the rotate-half permutation
                a_sb, b_sb = ab_t[j]
                rps = ps_misc.tile([P, CH], dt.float32, tag="rot")
                nc.tensor.matmul(rps, lhsT=pt_sb, rhs=raw, start=True, stop=True)
                t1 = work.tile([P, CH], dt.bfloat16, tag="t1", bufs=2)
                nc.gpsimd.tensor_tensor(out=t1, in0=raw, in1=a_sb, op=AluOpType.mult)
                t2 = work.tile([P, CH], dt.bfloat16, tag="t2", bufs=2)
                nc.vector.tensor_tensor(out=t2, in0=rps, in1=b_sb, op=AluOpType.mult)
                nc.vector.tensor_tensor(out=dest, in0=t1, in1=t2, op=AluOpType.add)

            def qk_tile(j, qk, h):
                # one head's q or k projection (+ pipelined rope of the
                # previous tile)
                cols = ds(j * CH, CH)
                wsrc = wq_r if qk == 0 else wk_r
                bsrc = bq_sb if qk == 0 else bk_sb
                xc = xc_t[j]
                ps = ps_acc.tile([P, CH], dt.float32, tag="acc")
                for cth in range(2):
                    wt = wtp.tile([P, NCT // 2, D], dt.bfloat16, tag="wt", name="wt")
                    nc.sync.dma_start(
                        out=wt, in_=wsrc[:, ds(cth * 8, 8), ds(h * D, D)]
                    )
                    for c in range(NCT // 2):
                        ct = cth * 8 + c
                        nc.tensor.matmul(
                            ps,
                            lhsT=wt[:, c, :],
                            rhs=xc[:, ct, :],
                            start=(ct == 0),
                            stop=(ct == NCT - 1),
                        )
                raw = work.tile([P, CH], dt.bfloat16, tag="raw", bufs=2)
                nc.vector.tensor_tensor(
                    out=raw,
                    in0=ps,
                    in1=bsrc[:, ds(h, 1)].to_broadcast([P, CH]),
                    op=AluOpType.add,
                )
                if qk == 0:
                    dest = qpp.tile([P, CH], dt.bfloat16, tag="qp")
                    qp_t[j].append(dest)
                else:
                    dest = kT[:, h, cols]
                if rope_pend[j] is not None:
                    emit_rope(j, *rope_pend[j])
                rope_pend[j] = (raw, dest)

            def qk_flush(j):
                if rope_pend[j] is not None:
                    emit_rope(j, *rope_pend[j])
                    rope_pend[j] = None

            def v_group(j, half, tt):
                # one [t-tile x half-of-heads] v projection group, bias
                # fused into the DVE copy out of PSUM
                xc = xc_t[j]
                ps = ps_acc.tile([P, CH], dt.float32, tag="acc")
                for ct in range(NCT):
                    nc.tensor.matmul(
                        ps,
                        lhsT=xc[:, ct, ds(tt * D, D)],
                        rhs=wv_sb[:, ct, ds(half * CH, CH)],
                        start=(ct == 0),
                        stop=(ct == NCT - 1),
                    )
                ti = 4 * j + tt
                nc.vector.tensor_tensor(
                    out=vS[:, ds(half * 4, 4), ds(ti * D, D)],
                    in0=ps[:, :].rearrange("p (a b) -> p a b", b=D),
                    in1=bv_rep[:, ds(half * CH, CH)].rearrange(
                        "p (a b) -> p a b", b=D
                    ),
                    op=AluOpType.add,
                )

            def att_head(j, h):
                if h == 0:
                    yc_t[j] = ycp.tile(
                        [P, HPC, CH], dt.bfloat16, tag="yc", name="yc"
                    )
                qp = qp_t[j][h]
                den_a = denp.tile([P, CH], dt.float32, tag="dena")
                yps = ps_y.tile([P, CH], dt.float32, tag="y")
                ntk = 4 * (j + 1)
                exq = []  # (ex, i, off) pending y-matmuls
                for i in range(ntk):
                    sps = ps_s.tile([P, CH], dt.float32, tag="s")
                    m = i - 4 * j
                    off = max(m, 0) * D  # valid tq cols start here
                    w = CH - off
                    nc.tensor.matmul(
                        sps[:, ds(off, w)],
                        lhsT=kT[:, h, ds(i * D, D)],
                        rhs=qp[:, ds(off, w)],
                        start=True,
                        stop=True,
                    )
                    ex = work.tile([P, CH], dt.bfloat16, tag="ex", bufs=3)
                    nc.scalar.activation(
                        ex[:, ds(off, w)], sps[:, ds(off, w)],
                        AF.Exp, scale=SCALE,
                    )
                    if m >= 0:
                        # triangular mask on the diagonal 128-block
                        nc.vector.tensor_tensor(
                            out=ex[:, ds(off, D)],
                            in0=ex[:, ds(off, D)],
                            in1=mask_sb,
                            op=AluOpType.mult,
                        )
                    if i == 0:
                        nc.vector.tensor_copy(
                            out=den_a[:, ds(off, w)], in_=ex[:, ds(off, w)]
                        )
                        if off > 0:
                            nc.vector.memset(den_a[:, ds(0, off)], 0.0)
                    else:
                        nc.vector.tensor_tensor(
                            out=den_a[:, ds(off, w)],
                            in0=den_a[:, ds(off, w)],
                            in1=ex[:, ds(off, w)],
                            op=AluOpType.add,
                        )
                    exq.append((ex, i, off))
                    if len(exq) > 2:
                        pex, pi, poff = exq.pop(0)
                        nc.tensor.matmul(
                            yps[:, ds(poff, CH - poff)],
                            lhsT=vS[:, h, ds(pi * D, D)],
                            rhs=pex[:, ds(poff, CH - poff)],
                            start=(pi == 0),
                            stop=False,
                        )
                while exq:
                    pex, pi, poff = exq.pop(0)
                    nc.tensor.matmul(
                        yps[:, ds(poff, CH - poff)],
                        lhsT=vS[:, h, ds(pi * D, D)],
                        rhs=pex[:, ds(poff, CH - poff)],
                        start=(pi == 0),
                        stop=(not exq),
                    )
                # denominator: partition-sum on the PE (ones-matmul), tiny
                # reciprocal on DVE (waits only on the PE), broadcast +
                # normalize on GpSimd. No engine queue ever waits on a slow
                # cross-engine chain here.
                dps = ps_s.tile([1, CH], dt.float32, tag="s", name="dps")
                nc.tensor.matmul(dps, lhsT=onc_sb, rhs=den_a, start=True,
                                 stop=True)
                yraw = work.tile([P, CH], dt.bfloat16, tag="yraw", bufs=2)
                nc.vector.tensor_copy(out=yraw, in_=yps)
                rcr = denp.tile([1, CH], dt.bfloat16, tag="rcr")
                with nc.allow_low_precision(
                    reason="softmax denominator reciprocal broadcast in bf16"
                ):
                    nc.vector.reciprocal(rcr, dps)
                rbc = denp.tile([P, CH], dt.bfloat16, tag="rbc")
                nc.gpsimd.partition_broadcast(rbc, rcr, channels=P)
                nc.gpsimd.tensor_tensor(
                    out=yc_t[j][:, h, :], in0=yraw, in1=rbc, op=AluOpType.mult
                )

            def oproj_piece(j, oc):
                yc = yc_t[j]
                wpc = []
                for hg in range(2):
                    w = wpp.tile([P, HPC // 2, CH], dt.bfloat16, tag="wpc",
                                 name="wpc", bufs=3)
                    nc.sync.dma_start(
                        out=w, in_=wp_r[:, ds(hg * 4, 4), ds(oc * CH, CH)]
                    )
                    wpc.append(w)
                for ttl in range(4):
                    ps = ps_misc.tile([P, CH], dt.float32, tag="rot")
                    for hc in range(HPC):
                        nc.tensor.matmul(
                            ps,
                            lhsT=yc[:, hc, ds(ttl * D, D)],
                            rhs=wpc[hc // 4][:, hc % 4, :],
                            start=(hc == 0),
                            stop=(hc == HPC - 1),
                        )
                    ot = outp.tile([P, CH], dt.float32, tag="ot")
                    nc.vector.tensor_tensor(
                        out=ot, in0=ps, in1=bp_rep[:, ds(oc * CH, CH)],
                        op=AluOpType.add,
                    )
                    nc.sync.dma_start(
                        out=out[ds((4 * j + ttl) * P, P), ds(oc * CH, CH)],
                        in_=ot,
                    )

            def proj_fillers(j):
                # list of closures emitting chunk j's projections piecewise
                fs = []
                for qk in range(2):
                    for h in range(HPC):
                        fs.append(lambda j=j, qk=qk, h=h: qk_tile(j, qk, h))
                fs.append(lambda j=j: qk_flush(j))
                for half in range(2):
                    for tt in range(4):
                        fs.append(
                            lambda j=j, half=half, tt=tt: v_group(j, half, tt)
                        )
                return fs

            # ---- prologue: chunk 0 inputs + projections ----
            # big constant / weight loads are emitted after the first few
            # projection tiles so the first matmuls aren't queued behind
            # them on the DMA rings
            load_chunk_inputs(0)
            p0 = proj_fillers(0)
            for f in p0[:4]:
                f()
            if NCH > 1:
                load_chunk_inputs(1)
            nc.sync.dma_start(
                out=bv_rep, in_=bv[0][None, :].to_broadcast([P, HPC * D])
            )
            nc.sync.dma_start(
                out=bp_rep, in_=bp[0][None, :].to_broadcast([P, C])
            )
            for cg in range(4):
                nc.sync.dma_start(
                    out=wv_sb[:, ds(cg * 4, 4), :],
                    in_=wv_r[:, ds(cg * 4, 4), :],
                )
            for f in p0[4:]:
                f()

            # ---- main software-pipelined loop ----
            for j in range(NCH):
                fillers = []
                if j + 2 < NCH:
                    fillers.append(lambda j=j: load_chunk_inputs(j + 2))
                if j + 1 < NCH:
                    fillers += proj_fillers(j + 1)
                if j >= 1:
                    # spread the out-projection pieces through the filler list
                    step = max(1, len(fillers) // 4)
                    for oc in range(C // CH):
                        fillers.insert(
                            min((oc + 1) * step + oc, len(fillers)),
                            lambda j=j, oc=oc: oproj_piece(j - 1, oc),
                        )
                per = (len(fillers) + HPC - 1) // HPC
                fi = 0
                for h in range(HPC):
                    att_head(j, h)
                    for _ in range(per):
                        if fi < len(fillers):
                            fillers[fi]()
                            fi += 1
                while fi < len(fillers):
                    fillers[fi]()
                    fi += 1

            # ---- epilogue: last chunk's output projection ----
            for oc in range(C // CH):
                oproj_piece(NCH - 1, oc)

    nc.compile()
    return nc


def _rope_tables(T):
    inv_freq = (
        1.0 / (10000.0 ** (np.arange(0, D, 2, dtype=np.float32) / np.float32(D)))
    ).astype(np.float32)
    t = np.arange(T, dtype=np.float32)
    freqs = t[:, None] * inv_freq[None, :]
    emb = np.concatenate((freqs, freqs), axis=-1)
    cos = np.cos(emb).astype(np.float32)
    sin = np.sin(emb).astype(np.float32)
    A = np.ascontiguousarray((cos + sin).T).astype(BF16)
    Bt = np.ascontiguousarray((cos - sin).T).astype(BF16)
    return A, Bt


def _rot_pt():
    Pm = np.zeros((D, D), dtype=np.float32)
    for d in range(64):
        Pm[d, 2 * d + 1] = -1.0
        Pm[64 + d, 2 * d] = 1.0
    return np.ascontiguousarray(Pm.T).astype(BF16)


def _maskd():
    # maskd[p, c] = 0 where tq < tk within a diagonal 128x128 block
    row = np.arange(P)[:, None]
    col = np.arange(D)[None, :]
    return np.where(col < row, 0.0, 1.0).astype(BF16)


def make_in_maps(x, w_attn, b_attn, w_proj, b_proj, T=2048):
    A, Bt = _rope_tables(T)
    pt = _rot_pt()
    maskd = _maskd()
    in_maps = []
    for core in range(8):
        b, g = core // 2, core % 2
        gs = slice(g * 1024, (g + 1) * 1024)
        bp_eff = b_proj if g == 0 else np.zeros_like(b_proj)
        in_maps.append(
            {
                "xT": np.ascontiguousarray(x[b][:T].T).astype(BF16),
                "wq": np.ascontiguousarray(w_attn[gs, :].T).astype(BF16),
                "wk": np.ascontiguousarray(w_attn[2048:4096][gs, :].T).astype(BF16),
                "wv": np.ascontiguousarray(w_attn[4096:6144][gs, :].T).astype(BF16),
                "wp": np.ascontiguousarray(w_proj[:, gs].T).astype(BF16),
                "ab_a": A,
                "ab_b": Bt,
                "bq": np.ascontiguousarray(
                    b_attn[gs].reshape(HPC, D).T
                ).astype(np.float32),
                "bk": np.ascontiguousarray(
                    b_attn[2048:4096][gs].reshape(HPC, D).T
                ).astype(np.float32),
                "bv": b_attn[4096:6144][gs].reshape(1, HPC * D).astype(BF16),
                "bp": bp_eff.reshape(1, C).astype(BF16),
                "maskd": maskd,
                "pt": pt,
                "onc": np.ones((P, 1), dtype=np.float32),
            }
        )
    return in_maps


_NC_CACHE = {}


def run(x, w_attn, b_attn, w_proj, b_proj, trace=False, trace_cores=None):
    T = x.shape[1]
    if T not in _NC_CACHE:
        _NC_CACHE[T] = build_nc(T)
    nc = _NC_CACHE[T]
    in_maps = make_in_maps(
        np.asarray(x, dtype=np.float32),
        np.asarray(w_attn, dtype=np.float32),
        np.asarray(b_attn, dtype=np.float32),
        np.asarray(w_proj, dtype=np.float32),
        np.asarray(b_proj, dtype=np.float32),
        T=T,
    )
    res = run_bass_kernel_spmd(
        nc, in_maps, core_ids=list(range(8)), trace=trace, trace_cores=trace_cores
    )
    T_, C_ = in_maps[0]["xT"].shape[1], C
    out = np.zeros((B, T_, C_), dtype=np.float32)
    for b in range(B):
        out[b] = res.results[2 * b]["out"] + res.results[2 * b + 1]["out"]
    return out, res


def kernel(x, w_attn, b_attn, w_proj, b_proj):
    out, _ = run(x, w_attn, b_attn, w_proj, b_proj, trace=False)
    return out


# revision 26
# speedup vs baseline: 1.4784x; 1.3146x over previous
"""Trainium2 Bass kernel for CausalSelfAttention (B=4, T=2048, C=2048, H=16).

Sharding: 8 cores = 4 batches x 2 head-groups (8 heads each).
Each core computes q/k/v projections for its heads, RoPE, causal attention,
and a partial output projection (row-parallel c_proj over its heads' columns).
Host sums the two partials per batch (standard row-parallel TP unshard).

On-chip layout notes:
  - All matmul contractions run with the contracted dim on partitions.
  - Host pre-transposes x and weights so every DMA is contiguous.
  - Scores are computed transposed (s^T[tk, tq]) so softmax normalization
    becomes: partition-sum via ones-matmul + reciprocal + DMA-replicate
    broadcast, and att@v needs no on-chip transposes at all.
  - RoPE rotate-half is a fixed 128x128 signed permutation applied via one
    extra matmul per q/k tile; cos/sin enter as elementwise tables.
  - Biases are added on DVE from partition-replicated SBUF tables instead
    of ones-matmuls on the PE.
  - The output projection of chunk j-1 is emitted inside chunk j, so the
    Tile scheduler can use it as ready PE filler while the scalar engine
    works through the exp() stream (which paces the attention windows),
    and the final DMA-out is spread across the kernel.
"""

import numpy as np
import ml_dtypes

import concourse.bass as bass
import concourse.mybir as mybir
import concourse.tile as tile
from concourse import bacc
from concourse.alu_op_type import AluOpType
from concourse.bass import ds
from concourse.bass_utils import run_bass_kernel_spmd

BF16 = ml_dtypes.bfloat16
F32 = np.float32

B = 4
C = 2048
H = 16
D = 128
HPC = 8          # heads per core
P = 128
CH = 512         # tq chunk width
NCT = C // P     # 16 contraction tiles
AF = mybir.ActivationFunctionType
SCALE = 1.0 / float(np.sqrt(np.float32(D)))


def build_nc(T=2048):
    NCH = T // CH
    dt = mybir.dt
    nc = bacc.Bacc(None, target_bir_lowering=False)

    xT = nc.dram_tensor("xT", [C, T], dt.bfloat16, kind="ExternalInput")
    wq = nc.dram_tensor("wq", [C, HPC * D], dt.bfloat16, kind="ExternalInput")
    wk = nc.dram_tensor("wk", [C, HPC * D], dt.bfloat16, kind="ExternalInput")
    wv = nc.dram_tensor("wv", [C, HPC * D], dt.bfloat16, kind="ExternalInput")
    wp = nc.dram_tensor("wp", [HPC * D, C], dt.bfloat16, kind="ExternalInput")
    ab_a = nc.dram_tensor("ab_a", [D, T], dt.bfloat16, kind="ExternalInput")
    ab_b = nc.dram_tensor("ab_b", [D, T], dt.bfloat16, kind="ExternalInput")
    bq = nc.dram_tensor("bq", [D, HPC], dt.float32, kind="ExternalInput")
    bk = nc.dram_tensor("bk", [D, HPC], dt.float32, kind="ExternalInput")
    bv = nc.dram_tensor("bv", [1, HPC * D], dt.bfloat16, kind="ExternalInput")
    bp = nc.dram_tensor("bp", [1, C], dt.bfloat16, kind="ExternalInput")
    maskd = nc.dram_tensor("maskd", [P, D], dt.bfloat16, kind="ExternalInput")
    pt = nc.dram_tensor("pt", [D, D], dt.bfloat16, kind="ExternalInput")
    onc = nc.dram_tensor("onc", [P, 1], dt.float32, kind="ExternalInput")
    out = nc.dram_tensor("out", [T, C], dt.float32, kind="ExternalOutput")
    scratch = nc.dram_tensor("den_scratch", [NCH, HPC, CH], dt.float32)

    xT_r = xT.rearrange("(ct p) t -> p ct t", p=P)
    wq_r = wq.rearrange("(ct p) d -> p ct d", p=P)
    wk_r = wk.rearrange("(ct p) d -> p ct d", p=P)
    wv_r = wv.rearrange("(ct p) d -> p ct d", p=P)
    wp_r = wp.rearrange("(hc p) o -> p hc o", p=P)

    with tile.TileContext(nc) as tc:
        with (
            tc.tile_pool(name="consts", bufs=1) as consts,
            tc.tile_pool(name="keep", bufs=1) as keep,
        ):
            mask_sb = consts.tile([P, D], dt.bfloat16)
            pt_sb = consts.tile([D, D], dt.bfloat16)
            bq_sb = consts.tile([D, HPC], dt.float32)
            bk_sb = consts.tile([D, HPC], dt.float32)
            bv_rep = consts.tile([P, HPC * D], dt.bfloat16)
            bp_rep = consts.tile([P, C], dt.bfloat16)
            onc_sb = consts.tile([P, 1], dt.float32)

            def load_consts():
                nc.sync.dma_start(out=mask_sb, in_=maskd[:])
                nc.sync.dma_start(out=pt_sb, in_=pt[:])
                nc.sync.dma_start(out=bq_sb, in_=bq[:])
                nc.sync.dma_start(out=bk_sb, in_=bk[:])
                nc.sync.dma_start(out=onc_sb, in_=onc[:])
                nc.sync.dma_start(
                    out=bv_rep, in_=bv[0][None, :].to_broadcast([P, HPC * D])
                )
                nc.sync.dma_start(
                    out=bp_rep, in_=bp[0][None, :].to_broadcast([P, C])
                )

            yT = keep.tile([P, HPC, T], dt.bfloat16)

            with (
                tc.tile_pool(name="kv", bufs=1) as kvp,
                tc.tile_pool(name="xw", bufs=1) as xwp,
                tc.tile_pool(name="wtp", bufs=2) as wtp,
                tc.tile_pool(name="wvp", bufs=1) as wvp,
                tc.tile_pool(name="wpp", bufs=3) as wpp,
                tc.tile_pool(name="work", bufs=4) as work,
                tc.tile_pool(name="qpp", bufs=9) as qpp,
                tc.tile_pool(name="denp", bufs=2) as denp,
                tc.tile_pool(name="outp", bufs=2) as outp,
                tc.tile_pool(name="ps_acc", bufs=2, space="PSUM") as ps_acc,
                tc.tile_pool(name="ps_misc", bufs=2, space="PSUM") as ps_misc,
                tc.tile_pool(name="ps_s", bufs=2, space="PSUM") as ps_s,
                tc.tile_pool(name="ps_y", bufs=2, space="PSUM") as ps_y,
            ):
                kT = kvp.tile([P, HPC, T], dt.bfloat16)
                vS = kvp.tile([P, HPC, T], dt.bfloat16)

                def oproj_chunk(jj):
                    # output projection for chunk jj's t-rows; emitted a
                    # chunk late so it serves as ready PE filler during the
                    # exp-paced attention windows
                    for oc in range(C // CH):
                        wpc = []
                        for hg in range(2):
                            w = wpp.tile([P, HPC // 2, CH], dt.bfloat16,
                                         tag="wpc", name="wpc")
                            nc.sync.dma_start(
                                out=w,
                                in_=wp_r[:, ds(hg * 4, 4), ds(oc * CH, CH)],
                            )
                            wpc.append(w)
                        for ttl in range(4):
                            tt = 4 * jj + ttl
                            ps = ps_misc.tile([P, CH], dt.float32, tag="rot",
                                              name="ops")
                            for hc in range(HPC):
                                nc.tensor.matmul(
                                    ps,
                                    lhsT=yT[:, hc, ds(tt * D, D)],
                                    rhs=wpc[hc // 4][:, hc % 4, :],
                                    start=(hc == 0),
                                    stop=(hc == HPC - 1),
                                )
                            ot = outp.tile([P, CH], dt.float32, tag="ot")
                            nc.vector.tensor_tensor(
                                out=ot, in0=ps,
                                in1=bp_rep[:, ds(oc * CH, CH)],
                                op=AluOpType.add,
                            )
                            nc.sync.dma_start(
                                out=out[ds(tt * P, P), ds(oc * CH, CH)],
                                in_=ot,
                            )

                for j in range(NCH):
                    cols = ds(j * CH, CH)
                    xc = xwp.tile([P, NCT, CH], dt.bfloat16, tag="xc")
                    for cg in range(4):
                        nc.sync.dma_start(
                            out=xc[:, ds(cg * 4, 4), :],
                            in_=xT_r[:, ds(cg * 4, 4), cols],
                        )
                    if j == 0:
                        load_consts()
                    a_sb = work.tile([D, CH], dt.bfloat16, tag="abA", bufs=2)
                    nc.sync.dma_start(out=a_sb, in_=ab_a[:, cols])
                    b_sb = work.tile([D, CH], dt.bfloat16, tag="abB", bufs=2)
                    nc.sync.dma_start(out=b_sb, in_=ab_b[:, cols])

                    qp_tiles = []

                    def emit_rope(raw, dest):
                        # q'/k' = A (.) raw + B (.) (P @ raw), via one PE
                        # matmul for the rotate-half permutation
                        rps = ps_misc.tile([P, CH], dt.float32, tag="rot")
                        nc.tensor.matmul(
                            rps, lhsT=pt_sb, rhs=raw, start=True, stop=True
                        )
                        t1 = work.tile([P, CH], dt.bfloat16, tag="t1", bufs=2)
                        nc.gpsimd.tensor_tensor(
                            out=t1, in0=raw, in1=a_sb, op=AluOpType.mult
                        )
                        t2 = work.tile([P, CH], dt.bfloat16, tag="t2", bufs=2)
                        nc.vector.tensor_tensor(
                            out=t2, in0=rps, in1=b_sb, op=AluOpType.mult
                        )
                        nc.vector.tensor_tensor(
                            out=dest, in0=t1, in1=t2, op=AluOpType.add
                        )

                    pending = None  # one-deep pipeline so rot never stalls PE
                    for qk in range(2):
                        wsrc = wq_r if qk == 0 else wk_r
                        bsrc = bq_sb if qk == 0 else bk_sb
                        for h in range(HPC):
                            wt = wtp.tile([P, NCT, D], dt.bfloat16, tag="wt")
                            nc.sync.dma_start(out=wt, in_=wsrc[:, :, ds(h * D, D)])
                            ps = ps_acc.tile([P, CH], dt.float32, tag="acc")
                            for ct in range(NCT):
                                nc.tensor.matmul(
                                    ps,
                                    lhsT=wt[:, ct, :],
                                    rhs=xc[:, ct, :],
                                    start=(ct == 0),
                                    stop=(ct == NCT - 1),
                                )
                            raw = work.tile([P, CH], dt.bfloat16, tag="raw", bufs=3)
                            nc.vector.tensor_tensor(
                                out=raw,
                                in0=ps,
                                in1=bsrc[:, ds(h, 1)].to_broadcast([P, CH]),
                                op=AluOpType.add,
                            )
                            if qk == 0:
                                dest = qpp.tile([P, CH], dt.bfloat16, tag="qp")
                                qp_tiles.append(dest)
                            else:
                                dest = kT[:, h, cols]
                            if pending is not None:
                                emit_rope(*pending)
                            pending = (raw, dest)
                    emit_rope(*pending)

                    for half in range(2):
                        wvt = wvp.tile([P, NCT, CH], dt.bfloat16, tag="wv")
                        nc.sync.dma_start(out=wvt, in_=wv_r[:, :, ds(half * CH, CH)])
                        for tt in range(4):
                            ps = ps_acc.tile([P, CH], dt.float32, tag="acc")
                            for ct in range(NCT):
                                nc.tensor.matmul(
                                    ps,
                                    lhsT=xc[:, ct, ds(tt * D, D)],
                                    rhs=wvt[:, ct, :],
                                    start=(ct == 0),
                                    stop=(ct == NCT - 1),
                                )
                            ti = 4 * j + tt
                            nc.vector.tensor_tensor(
                                out=vS[:, ds(half * 4, 4), ds(ti * D, D)],
                                in0=ps[:, :].rearrange("p (a b) -> p a b", b=D),
                                in1=bv_rep[:, ds(half * CH, CH)].rearrange(
                                    "p (a b) -> p a b", b=D
                                ),
                                op=AluOpType.add,
                            )

                    # chunk j-1's output projection: ready PE filler for
                    # the attention windows below
                    if j >= 1:
                        oproj_chunk(j - 1)

                    den_rows = denp.tile([HPC, CH], dt.float32, tag="dr", bufs=1)
                    yraw_tiles = []
                    for h in range(HPC):
                        qp = qp_tiles[h]
                        den_a = denp.tile([P, CH], dt.float32, tag="dena")
                        yps = ps_y.tile([P, CH], dt.float32, tag="y")
                        ntk = 4 * (j + 1)
                        exq = []  # (ex, i, off) pending y-matmuls
                        for i in range(ntk):
                            sps = ps_s.tile([P, CH], dt.float32, tag="s")
                            m = i - 4 * j
                            off = max(m, 0) * D  # valid tq cols start here
                            w = CH - off
                            nc.tensor.matmul(
                                sps[:, ds(off, w)],
                                lhsT=kT[:, h, ds(i * D, D)],
                                rhs=qp[:, ds(off, w)],
                                start=True,
                                stop=True,
                            )
                            ex = work.tile([P, CH], dt.bfloat16, tag="ex", bufs=4)
                            nc.scalar.activation(
                                ex[:, ds(off, w)], sps[:, ds(off, w)],
                                AF.Exp, scale=SCALE,
                            )
                            if m >= 0:
                                # triangular mask on the diagonal 128-block
                                nc.vector.tensor_tensor(
                                    out=ex[:, ds(off, D)],
                                    in0=ex[:, ds(off, D)],
                                    in1=mask_sb,
                                    op=AluOpType.mult,
                                )
                            if i == 0:
                                nc.vector.tensor_copy(
                                    out=den_a[:, ds(off, w)], in_=ex[:, ds(off, w)]
                                )
                                if off > 0:
                                    nc.vector.memset(den_a[:, ds(0, off)], 0.0)
                            else:
                                nc.vector.tensor_tensor(
                                    out=den_a[:, ds(off, w)],
                                    in0=den_a[:, ds(off, w)],
                                    in1=ex[:, ds(off, w)],
                                    op=AluOpType.add,
                                )
                            exq.append((ex, i, off))
                            if len(exq) > 2:
                                pex, pi, poff = exq.pop(0)
                                nc.tensor.matmul(
                                    yps[:, ds(poff, CH - poff)],
                                    lhsT=vS[:, h, ds(pi * D, D)],
                                    rhs=pex[:, ds(poff, CH - poff)],
                                    start=(pi == 0),
                                    stop=False,
                                )
                        while exq:
                            pex, pi, poff = exq.pop(0)
                            nc.tensor.matmul(
                                yps[:, ds(poff, CH - poff)],
                                lhsT=vS[:, h, ds(pi * D, D)],
                                rhs=pex[:, ds(poff, CH - poff)],
                                start=(pi == 0),
                                stop=(not exq),
                            )
                        yraw = work.tile([P, CH], dt.bfloat16, tag="yraw", bufs=9)
                        nc.vector.tensor_copy(out=yraw, in_=yps)
                        yraw_tiles.append(yraw)
                        dsum = ps_y.tile([1, CH], dt.float32, tag="y")
                        nc.tensor.matmul(
                            dsum, lhsT=onc_sb, rhs=den_a, start=True, stop=True
                        )
                        dstage = denp.tile([1, CH], dt.float32, tag="dstage")
                        nc.scalar.activation(dstage, dsum, AF.Copy)
                        nc.sync.dma_start(out=den_rows[ds(h, 1), :], in_=dstage)
                    # one wide reciprocal for all 8 heads, then per-head
                    # partition-broadcast via DMA replicate
                    rec8 = denp.tile([HPC, CH], dt.float32, tag="rec8", bufs=1)
                    nc.vector.reciprocal(rec8, den_rows)
                    nc.sync.dma_start(out=scratch[j], in_=rec8)
                    for h in range(HPC):
                        rbc = work.tile([P, CH], dt.float32, tag="rbc", bufs=2)
                        nc.sync.dma_start(
                            out=rbc,
                            in_=scratch[j, h][None, :].to_broadcast([P, CH]),
                        )
                        nc.gpsimd.tensor_tensor(
                            out=yT[:, h, cols],
                            in0=yraw_tiles[h],
                            in1=rbc,
                            op=AluOpType.mult,
                        )

                # last chunk's output projection
                oproj_chunk(NCH - 1)

    nc.compile()
    return nc


def _rope_tables(T):
    inv_freq = (
        1.0 / (10000.0 ** (np.arange(0, D, 2, dtype=np.float32) / np.float32(D)))
    ).astype(np.float32)
    t = np.arange(T, dtype=np.float32)
    freqs = t[:, None] * inv_freq[None, :]
    emb = np.concatenate((freqs, freqs), axis=-1)
    cos = np.cos(emb).astype(np.float32)
    sin = np.sin(emb).astype(np.float32)
    A = np.ascontiguousarray((cos + sin).T).astype(BF16)
    Bt = np.ascontiguousarray((cos - sin).T).astype(BF16)
    return A, Bt


def _rot_pt():
    Pm = np.zeros((D, D), dtype=np.float32)
    for d in range(64):
        Pm[d, 2 * d + 1] = -1.0
        Pm[64 + d, 2 * d] = 1.0
    return np.ascontiguousarray(Pm.T).astype(BF16)


def _maskd():
    # maskd[p, c] = 0 where tq < tk within a diagonal 128x128 block
    row = np.arange(P)[:, None]
    col = np.arange(D)[None, :]
    return np.where(col < row, 0.0, 1.0).astype(BF16)


def make_in_maps(x, w_attn, b_attn, w_proj, b_proj, T=2048):
    A, Bt = _rope_tables(T)
    pt = _rot_pt()
    maskd = _maskd()
    onc = np.ones((P, 1), dtype=np.float32)
    in_maps = []
    for core in range(8):
        b, g = core // 2, core % 2
        gs = slice(g * 1024, (g + 1) * 1024)
        bp_eff = b_proj if g == 0 else np.zeros_like(b_proj)
        in_maps.append(
            {
                "xT": np.ascontiguousarray(x[b][:T].T).astype(BF16),
                "wq": np.ascontiguousarray(w_attn[gs, :].T).astype(BF16),
                "wk": np.ascontiguousarray(w_attn[2048:4096][gs, :].T).astype(BF16),
                "wv": np.ascontiguousarray(w_attn[4096:6144][gs, :].T).astype(BF16),
                "wp": np.ascontiguousarray(w_proj[:, gs].T).astype(BF16),
                "ab_a": A,
                "ab_b": Bt,
                "bq": np.ascontiguousarray(
                    b_attn[gs].reshape(HPC, D).T
                ).astype(np.float32),
                "bk": np.ascontiguousarray(
                    b_attn[2048:4096][gs].reshape(HPC, D).T
                ).astype(np.float32),
                "bv": b_attn[4096:6144][gs].reshape(1, HPC * D).astype(BF16),
                "bp": bp_eff.reshape(1, C).astype(BF16),
                "maskd": maskd,
                "pt": pt,
                "onc": onc,
            }
        )
    return in_maps


_NC_CACHE = {}


def run(x, w_attn, b_attn, w_proj, b_proj, trace=False, trace_cores=None):
    T = x.shape[1]
    if T not in _NC_CACHE:
        _NC_CACHE[T] = build_nc(T)
    nc = _NC_CACHE[T]
    in_maps = make_in_maps(
        np.asarray(x, dtype=np.float32),
        np.asarray(w_attn, dtype=np.float32),
        np.asarray(b_attn, dtype=np.float32),
        np.asarray(w_proj, dtype=np.float32),
        np.asarray(b_proj, dtype=np.float32),
        T=T,
    )
    res = run_bass_kernel_spmd(
        nc, in_maps, core_ids=list(range(8)), trace=trace, trace_cores=trace_cores
    )
    T_, C_ = in_maps[0]["xT"].shape[1], C
    out = np.zeros((B, T_, C_), dtype=np.float32)
    for b in range(B):
        out[b] = res.results[2 * b]["out"] + res.results[2 * b + 1]["out"]
    return out, res


def kernel(x, w_attn, b_attn, w_proj, b_proj):
    out, _ = run(x, w_attn, b_attn, w_proj, b_proj, trace=False)
    return out
